# revision 1
# baseline (speedup 1.0000x reference)
"""v2 APPNP kernel: race-free striped-ELL formulation.

Key facts driving the design (HW-verified):
  - dma_gather layout: token t -> ms[t%128, t//128, :]; exact on HW.
  - dma_scatter_add RACES on duplicate indices within one instruction
    (descriptors fan out over 16 SDMA engines).  All scatters here use
    permutation (unique) indices only.

Per core, per window w (4 source windows of 25K nodes for int16 gather
indices), edges are laid out as a degree-sorted ELL:
  - dests sorted by in-window degree (desc) -> slot permutation perm_w
  - stripe j holds the j-th in-window edge of every dest with deg_w > j,
    at token position (stripe_base_j + slot)
  - static per-stripe caps (multiples of 128) shared by all cores; pad
    tokens are (gather idx 0, val 0)
Accumulation: acc_w[slot] += val * h_full[col] on DVE (copy for stripe 0),
then one dma_scatter_add per window drains acc_w to h_own with idx =
perm_w (unique -> race-free), adding onto h_own = alpha * h_old.
"""

from dataclasses import dataclass

import numpy as np


@dataclass(frozen=True)
class Cfg2:
    n: int = 100000
    fin: int = 512
    fhid: int = 256
    fout: int = 64
    alpha: float = 0.01
    k: int = 10
    ncores: int = 8
    nw: int = 4
    bin_tok: int = 12288      # max tokens per gather (q7 scratch: idxs int32 must fit 64KB)
    use_collective: bool = True   # debug: False replaces AG with local copy
    split_ag: bool = False        # 4 sub-allgathers overlapped with compute
    bf16_ag: bool = False         # exchange h in bf16 (no win: collective cost is fixed)

    @property
    def subsz(self):              # rows per sub-allgather slice
        return self.npc // self.nw

    @property
    def npc(self):
        return self.n // self.ncores

    @property
    def win(self):
        return self.n // self.nw

    @property
    def npc_pad(self):
        return ((self.npc + 127) // 128) * 128

    @property
    def slot_cols(self):
        return self.npc_pad // 128


CFG2 = Cfg2()

# Static stripe caps (slots per stripe, multiples of 128) for the seed-0
# problem instance; preprocess() verifies the actual data fits and
# recomputes if not (different caps -> different compiled graph).
DEFAULT_CAPS_SPLIT = (
    (12544, 12544, 12416, 12032, 11392, 10240, 8704, 7040, 5248, 3712, 2432,
     1536, 896, 512, 256, 128, 128, 128, 128, 128, 128, 128),
    (12544, 12544, 12416, 12032, 11392, 10240, 8832, 6912, 5248, 3712, 2432,
     1536, 896, 512, 256, 128, 128, 128, 128, 128, 128, 128, 128, 128),
    (12544, 12544, 12416, 12032, 11392, 10240, 8704, 7040, 5248, 3712, 2432,
     1536, 896, 512, 256, 128, 128, 128, 128, 128, 128, 128, 128),
    (12544, 12544, 12416, 12032, 11392, 10240, 8704, 7040, 5248, 3712, 2432,
     1536, 896, 512, 256, 128, 128, 128, 128, 128, 128, 128, 128),
)

DEFAULT_CAPS = (
    (12544, 12544, 12416, 12032, 11392, 10240, 8704, 6912, 5248, 3712, 2432,
     1536, 896, 512, 256, 128, 128, 128, 128, 128, 128, 128),
    (12544, 12544, 12416, 12032, 11392, 10240, 8832, 7040, 5248, 3712, 2432,
     1536, 896, 512, 384, 128, 128, 128, 128, 128, 128, 128, 128),
    (12544, 12544, 12416, 12032, 11392, 10240, 8704, 7040, 5248, 3712, 2432,
     1536, 896, 512, 256, 128, 128, 128, 128, 128, 128, 128, 128),
    (12544, 12544, 12416, 12032, 11392, 10240, 8704, 6912, 5248, 3712, 2432,
     1536, 896, 512, 256, 128, 128, 128, 128, 128, 128, 128, 128),
)


def window_of(cfg: Cfg2, g):
    """Window index of global source node g (array ok)."""
    if cfg.split_ag:
        return (g % cfg.npc) // cfg.subsz
    return g // cfg.win


def lidx_of(cfg: Cfg2, g):
    """Index of node g within its window's gather buffer."""
    if cfg.split_ag:
        return (g // cfg.npc) * cfg.subsz + (g % cfg.npc) % cfg.subsz
    return g % cfg.win


def plan_layout(cfg: Cfg2, caps):
    """Static token-stream layout derived from caps only.

    Returns dict with per-window stripe bases, bin list and per-bin
    DVE-add segments.  bins: list of (w, t0_global, ntok).  segments per
    bin: list of (acc_col0, ncols, bin_local_col0) in 128-token columns.
    """
    stripes = []   # (w, j, base_global, ntok)
    t = 0
    win_tok = []
    for w in range(cfg.nw):
        w0 = t
        for j, cap in enumerate(caps[w]):
            assert cap % 128 == 0
            stripes.append((w, j, t, cap))
            t += cap
        win_tok.append((w0, t - w0))
    total = t

    bins = []      # (w, t0, ntok, segments)
    for w in range(cfg.nw):
        w0, wlen = win_tok[w]
        pos = w0
        while pos < w0 + wlen:
            ntok = min(cfg.bin_tok, w0 + wlen - pos)
            # segments: intersect [pos, pos+ntok) with stripes of window w
            segs = []
            for (sw, j, base, scap) in stripes:
                if sw != w:
                    continue
                lo = max(pos, base)
                hi = min(pos + ntok, base + scap)
                if lo < hi:
                    # acc columns are 128-token units from stripe start
                    segs.append((
                        j,
                        (lo - base) // 128,        # acc col0
                        (hi - lo) // 128,          # ncols
                        (lo - pos) // 128,         # bin-local col0
                    ))
            bins.append((w, pos, ntok, segs))
            pos += ntok
    return {"total": total, "bins": bins, "win_tok": win_tok}


def build_graph2(cfg: Cfg2, caps):
    import concourse.bacc as bacc
    import concourse.mybir as mybir
    import concourse.tile as tile

    f32 = mybir.dt.float32
    bf16 = mybir.dt.bfloat16
    i16 = mybir.dt.int16
    KC = cfg.fin // 128
    MC = cfg.fhid // 128
    SC = cfg.slot_cols            # acc columns (npc_pad/128)
    lay = plan_layout(cfg, caps)
    total = lay["total"]

    nc = bacc.Bacc("TRN2", num_devices=cfg.ncores)

    xT = nc.dram_tensor("xT", [cfg.fin, cfg.npc_pad], f32, kind="ExternalInput")
    w1r = nc.dram_tensor("w1r", [128, KC, MC, 128], f32, kind="ExternalInput")
    b1r = nc.dram_tensor("b1r", [128, MC], f32, kind="ExternalInput")
    w2r = nc.dram_tensor("w2r", [128, MC, cfg.fout], f32, kind="ExternalInput")
    b2r = nc.dram_tensor("b2r", [1, cfg.fout], f32, kind="ExternalInput")
    gidx = nc.dram_tensor("gidx", [128, total // 16], i16, kind="ExternalInput")
    vals = nc.dram_tensor("vals", [128, total // 128], f32, kind="ExternalInput")
    sidx = nc.dram_tensor(
        "sidx", [cfg.nw, 128, cfg.npc_pad // 16], i16, kind="ExternalInput"
    )
    out = nc.dram_tensor("out", [cfg.npc, cfg.fout], f32, kind="ExternalOutput")

    h_own_a = nc.dram_tensor("h_own_a", [cfg.npc, cfg.fout], f32, kind="Internal")
    h_own_b = nc.dram_tensor("h_own_b", [cfg.npc, cfg.fout], f32, kind="Internal")
    if cfg.split_ag:
        h_sub = [
            nc.dram_tensor(
                f"h_sub{b}", [cfg.ncores * cfg.subsz, cfg.fout], f32,
                kind="Internal", addr_space="Shared",
            )
            for b in range(cfg.nw)
        ]
    else:
        h_full = nc.dram_tensor(
            "h_full", [cfg.n, cfg.fout], f32, kind="Internal", addr_space="Shared"
        )
    if cfg.bf16_ag:
        assert not cfg.split_ag
        # bf16 values packed 2-per-f32 element: the emulated collective
        # costs ~21ns per ELEMENT (independent of dtype/bytes), so wider
        # elements cut its cost 2x (u64 4x-packing crashes the emulation).
        h_bf_own = nc.dram_tensor(
            "h_bf_own", [cfg.npc, cfg.fout // 2], f32, kind="Internal"
        )
        h_bf_full = nc.dram_tensor(
            "h_bf_full", [cfg.n, cfg.fout // 2], f32, kind="Internal",
            addr_space="Shared",
        )

    flat = cfg.npc * cfg.fout // 128

    def flatv(t):
        return t[:].rearrange("a b -> (a b)").rearrange("(p q) -> p q", p=128)

    def flatv_bf(t):
        return t[:].bitcast(bf16).rearrange("a b -> (a b)").rearrange(
            "(p q) -> p q", p=128)

    out_flat = flatv(out)

    with tile.TileContext(nc) as tc:
        # ---------------- dense phase (own pools, closed after) ----------
        with tc.tile_pool(name="wpool", bufs=1) as wpool, \
             tc.tile_pool(name="xpool", bufs=2) as xpool, \
             tc.tile_pool(name="hpool", bufs=2) as hpool, \
             tc.tile_pool(name="opool", bufs=3) as opool, \
             tc.tile_pool(name="ppool", bufs=2, space="PSUM") as ppool, \
             tc.tile_pool(name="p2pool", bufs=2, space="PSUM") as p2pool:
            w1t = wpool.tile([128, KC, MC, 128], f32)
            nc.sync.dma_start(w1t[:], w1r[:])
            b1t = wpool.tile([128, MC], f32)
            nc.sync.dma_start(b1t[:], b1r[:])
            w2t = wpool.tile([128, MC, cfg.fout], f32)
            nc.sync.dma_start(w2t[:], w2r[:])
            b2t = wpool.tile([1, cfg.fout], f32)
            nc.sync.dma_start(b2t[:], b2r[:])
            onest = wpool.tile([1, 128], f32)
            nc.vector.memset(onest[:], 1.0)

            row0 = 0
            while row0 < cfg.npc:
                nsz = min(512, cfg.npc_pad - row0)
                xt = xpool.tile([128, KC, nsz], f32, tag="xt")
                nc.sync.dma_start(
                    xt[:],
                    xT[:, row0:row0 + nsz].rearrange("(kc p) n -> p kc n", p=128),
                )
                h1t = []
                for mc in range(MC):
                    ps = ppool.tile([128, nsz], f32, tag="ps")
                    for kc in range(KC):
                        nc.tensor.matmul(
                            ps[:], w1t[:, kc, mc, :], xt[:, kc, :],
                            start=(kc == 0), stop=(kc == KC - 1),
                        )
                    ht = hpool.tile([128, nsz], f32, tag=f"h1t{mc}")
                    nc.scalar.activation(
                        ht[:], ps[:], mybir.ActivationFunctionType.Relu,
                        bias=b1t[:, mc:mc + 1], scale=1.0,
                    )
                    h1t.append(ht)
                for ns in range(0, nsz, 128):
                    ssz = min(128, nsz - ns)
                    rows = min(ssz, cfg.npc - (row0 + ns))
                    if rows <= 0:
                        break
                    ps2 = p2pool.tile([128, cfg.fout], f32, tag="ps2")
                    for mc in range(MC):
                        nc.tensor.matmul(
                            ps2[:ssz, :], h1t[mc][:, ns:ns + ssz], w2t[:, mc, :],
                            start=(mc == 0), stop=False,
                        )
                    nc.tensor.matmul(
                        ps2[:ssz, :], onest[:, :ssz], b2t[:], start=False, stop=True
                    )
                    ho = opool.tile([128, cfg.fout], f32, tag="ho")
                    nc.scalar.activation(
                        ho[:ssz, :], ps2[:ssz, :], mybir.ActivationFunctionType.Copy
                    )
                    gr0 = row0 + ns
                    nc.sync.dma_start(h_own_a[gr0:gr0 + rows, :], ho[:rows, :])
                row0 += nsz

        # ---------------- propagation ----------------
        with tc.tile_pool(name="sxpool", bufs=1) as sxpool, \
             tc.tile_pool(name="apool", bufs=2) as apool, \
             tc.tile_pool(name="mpool", bufs=2) as mpool, \
             tc.tile_pool(name="ipool", bufs=6) as ipool, \
             tc.tile_pool(name="dqf", bufs=2) as dqf, \
             tc.tile_pool(name="dqb", bufs=2) as dqb:

            # resident scatter-permutation idx tiles (constant)
            sidx_t = []
            for w in range(cfg.nw):
                st = sxpool.tile([128, cfg.npc_pad // 16], i16, tag=f"sidx{w}")
                nc.sync.dma_start(st[:], sidx[w])
                sidx_t.append(st)

            for it in range(cfg.k):
                cur = h_own_a if it % 2 == 0 else h_own_b
                nxt = h_own_b if it % 2 == 0 else h_own_a
                if cfg.split_ag:
                    for b in range(cfg.nw):
                        nc.gpsimd.collective_compute(
                            "AllGather",
                            mybir.AluOpType.bypass,
                            replica_groups=[list(range(cfg.ncores))],
                            ins=[cur[b * cfg.subsz:(b + 1) * cfg.subsz, :]],
                            outs=[h_sub[b][:]],
                        )
                elif cfg.use_collective and cfg.bf16_ag:
                    # quantize cur -> bf16 (first iter reads the dense output;
                    # later iters write h_bf_own at drain time instead)
                    if it == 0:
                        for qc in range(2):
                            qf = dqf.tile([128, flat // 2], f32, tag="dqf")
                            nc.sync.dma_start(
                                qf[:], flatv(cur)[:, qc * (flat // 2):(qc + 1) * (flat // 2)]
                            )
                            qb = dqb.tile([128, flat // 2], bf16, tag="dqb")
                            nc.vector.tensor_copy(qb[:], qf[:])
                            nc.sync.dma_start(
                                flatv_bf(h_bf_own)[:, qc * (flat // 2):(qc + 1) * (flat // 2)],
                                qb[:],
                            )
                    nc.gpsimd.collective_compute(
                        "AllGather",
                        mybir.AluOpType.bypass,
                        replica_groups=[list(range(cfg.ncores))],
                        ins=[h_bf_own[:]],
                        outs=[h_bf_full[:]],
                    )
                    # dequantize h_bf_full -> h_full (f32) for the gathers
                    fl_all = cfg.n * cfg.fout // 128
                    nch = 16
                    for qc in range(nch):
                        w0q = qc * (fl_all // nch)
                        w1q = (qc + 1) * (fl_all // nch)
                        qb = dqb.tile([128, fl_all // nch], bf16, tag="dqb2")
                        nc.sync.dma_start(
                            qb[:],
                            h_bf_full[:].bitcast(bf16).rearrange("a b -> (a b)")
                            .rearrange("(p q) -> p q", p=128)[:, w0q:w1q],
                        )
                        qf = dqf.tile([128, fl_all // nch], f32, tag="dqf2")
                        nc.vector.tensor_copy(qf[:], qb[:])
                        nc.sync.dma_start(
                            h_full[:].rearrange("a b -> (a b)")
                            .rearrange("(p q) -> p q", p=128)[:, w0q:w1q],
                            qf[:],
                        )
                elif cfg.use_collective:
                    nc.gpsimd.collective_compute(
                        "AllGather",
                        mybir.AluOpType.bypass,
                        replica_groups=[list(range(cfg.ncores))],
                        ins=[cur[:]],
                        outs=[h_full[:]],
                    )
                else:
                    for qc in range(2):
                        sl = slice(qc * (flat // 2), (qc + 1) * (flat // 2))
                        hb = dqf.tile([128, flat // 2], f32, tag="dqf")
                        nc.sync.dma_start(hb[:], flatv(cur)[:, sl])
                        nc.sync.dma_start(
                            h_full[:cfg.npc, :].rearrange("a b -> (a b)")
                            .rearrange("(p q) -> p q", p=128)[:, sl],
                            hb[:],
                        )
                # nxt <- alpha * cur  (overlaps the collective: reads cur,
                # writes the other buffer)
                for qc in range(2):
                    sl = slice(qc * (flat // 2), (qc + 1) * (flat // 2))
                    ht = dqf.tile([128, flat // 2], f32, tag="dqf")
                    nc.sync.dma_start(ht[:], flatv(cur)[:, sl])
                    nc.vector.tensor_scalar_mul(ht[:], ht[:], float(cfg.alpha))
                    nc.sync.dma_start(flatv(nxt)[:, sl], ht[:])

                accs = {}
                for (w, t0, ntok, segs) in lay["bins"]:
                    if w not in accs:
                        acc = apool.tile([128, SC, cfg.fout], f32, tag="acc")
                        accs[w] = acc
                        # stripe 0 covers caps[w][0]//128 cols via copies;
                        # zero any remaining cols (zero-degree dests).
                        m0 = (caps[w][0] // 128) if caps[w] else 0
                        if m0 < SC:
                            nc.vector.memset(acc[:, m0:, :], 0.0)
                    acc = accs[w]
                    nb = ntok // 128
                    gi = ipool.tile([128, cfg.bin_tok // 16], i16, tag="gi")
                    nc.sync.dma_start(
                        gi[:, :ntok // 16], gidx[:, t0 // 16:(t0 + ntok) // 16]
                    )
                    vt = ipool.tile([128, cfg.bin_tok // 128], f32, tag="vt")
                    nc.sync.dma_start(
                        vt[:, :nb], vals[:, t0 // 128:(t0 + ntok) // 128]
                    )
                    ms = mpool.tile([128, cfg.bin_tok // 128, cfg.fout], f32, tag="ms")
                    gsrc = h_sub[w][:, :] if cfg.split_ag else \
                        h_full[w * cfg.win:(w + 1) * cfg.win, :]
                    nc.gpsimd.dma_gather(
                        ms[:, :nb, :],
                        gsrc,
                        gi[:, :ntok // 16],
                        ntok,
                        ntok,
                        cfg.fout,
                        single_packet=False,
                    )
                    nc.vector.tensor_mul(
                        ms[:, :nb, :],
                        ms[:, :nb, :],
                        vt[:, :nb].unsqueeze(2).broadcast_to((128, nb, cfg.fout)),
                    )
                    for (j, c0, ncols, b0) in segs:
                        src = ms[:, b0:b0 + ncols, :]
                        dst = acc[:, c0:c0 + ncols, :]
                        if j == 0:
                            nc.vector.tensor_copy(dst, src)
                        else:
                            nc.vector.tensor_add(dst, dst, src)
                    # (drain below; nxt quantization happens after drains)
                    # after last bin of window w: drain (split into halves —
                    # scatter_add emits 2 tx descriptors per token and the
                    # SWDGE ring carveout is 1024/engine, so one instruction
                    # must stay <= ~8k tokens)
                    if t0 + ntok == lay["win_tok"][w][0] + lay["win_tok"][w][1]:
                        half = cfg.npc_pad // 2
                        assert half % 128 == 0
                        for hh in range(2):
                            lo = hh * half
                            nvalid = min(max(cfg.npc - lo, 0), half)
                            if nvalid == 0:
                                continue
                            nc.gpsimd.dma_scatter_add(
                                nxt[:],
                                acc[:, lo // 128:(lo + half) // 128, :],
                                sidx_t[w][:, lo // 16:(lo + half) // 16],
                                half,
                                nvalid,
                                cfg.fout,
                                single_packet=False,
                            )
                if cfg.use_collective and cfg.bf16_ag and it + 1 < cfg.k:
                    for qc in range(2):
                        qf = dqf.tile([128, flat // 2], f32, tag="dqf")
                        nc.sync.dma_start(
                            qf[:], flatv(nxt)[:, qc * (flat // 2):(qc + 1) * (flat // 2)]
                        )
                        qb = dqb.tile([128, flat // 2], bf16, tag="dqb")
                        nc.vector.tensor_copy(qb[:], qf[:])
                        nc.sync.dma_start(
                            flatv_bf(h_bf_own)[:, qc * (flat // 2):(qc + 1) * (flat // 2)],
                            qb[:],
                        )

            fin = h_own_b if cfg.k % 2 == 1 else h_own_a
            if cfg.k == 0:
                fin = h_own_a
            for qc in range(2):
                sl = slice(qc * (flat // 2), (qc + 1) * (flat // 2))
                ot = dqf.tile([128, flat // 2], f32, tag="dqf")
                nc.sync.dma_start(ot[:], flatv(fin)[:, sl])
                nc.sync.dma_start(out_flat[:, sl], ot[:])

    return nc


# ---------------------------------------------------------------------------
# Host preprocessing
# ---------------------------------------------------------------------------

def _wrap16(a):
    a = a.reshape(-1, 16).T
    return np.ascontiguousarray(np.tile(a, (8, 1)).astype(np.int16))


def _wrap128(a):
    return np.ascontiguousarray(a.reshape(-1, 128).T)


def compute_caps(cfg: Cfg2, row, col):
    caps = []
    wn_all = window_of(cfg, col)
    for w in range(cfg.nw):
        maxlen = 0
        njs = []
        for c in range(cfg.ncores):
            m = (row // cfg.npc == c) & (wn_all == w)
            deg = np.bincount(row[m] - c * cfg.npc, minlength=cfg.npc)
            mx = int(deg.max())
            nj = [(deg > j).sum() for j in range(mx)]
            njs.append(nj)
            maxlen = max(maxlen, mx)
        tab = []
        for j in range(maxlen):
            mm = max(int(nj[j]) if j < len(nj) else 0 for nj in njs)
            tab.append(-(-mm // 128) * 128)
        caps.append(tuple(tab))
    return tuple(caps)


def caps_fit(cfg: Cfg2, caps, row, col):
    try:
        if any(c > cfg.npc_pad or c <= 0 for tab in caps for c in tab):
            return False
        wn_all = window_of(cfg, col)
        for w in range(cfg.nw):
            for c in range(cfg.ncores):
                m = (row // cfg.npc == c) & (wn_all == w)
                deg = np.bincount(row[m] - c * cfg.npc, minlength=cfg.npc)
                mx = int(deg.max())
                if mx > len(caps[w]):
                    return False
                nj = np.array([(deg > j).sum() for j in range(mx)])
                if (nj > np.array(caps[w][:mx])).any():
                    return False
        return True
    except Exception:
        return False


def preprocess2(cfg: Cfg2, caps, x, row, col, val, W1, b1, W2, b2):
    KC = cfg.fin // 128
    MC = cfg.fhid // 128
    lay = plan_layout(cfg, caps)
    total = lay["total"]

    x = np.asarray(x, np.float32)
    row = np.asarray(row, np.int64)
    col = np.asarray(col, np.int64)
    val = np.asarray(val, np.float32)
    W1 = np.asarray(W1, np.float32)
    b1 = np.asarray(b1, np.float32)
    W2 = np.asarray(W2, np.float32)
    b2 = np.asarray(b2, np.float32)

    w1r = np.ascontiguousarray(W1.reshape(KC, 128, MC, 128).transpose(1, 0, 2, 3))
    b1r = np.ascontiguousarray(b1.reshape(MC, 128).T)
    w2r = np.ascontiguousarray(W2.reshape(MC, 128, cfg.fout).transpose(1, 0, 2))
    b2r = np.ascontiguousarray(b2.reshape(1, cfg.fout))

    stripe_base = {}
    t = 0
    for w in range(cfg.nw):
        for j, cap in enumerate(caps[w]):
            stripe_base[(w, j)] = t
            t += cap
    assert t == total

    in_maps = []
    valp = val * np.float32(1.0 - cfg.alpha)
    for c in range(cfg.ncores):
        cm = row // cfg.npc == c
        r_c = row[cm] - c * cfg.npc
        c_c = col[cm]
        v_c = valp[cm]

        g_stream = np.zeros(total, np.int16)
        v_stream = np.zeros(total, np.float32)
        sidx_all = np.full((cfg.nw, cfg.npc_pad), -1, np.int64)

        wn_c = window_of(cfg, c_c)
        li_c = lidx_of(cfg, c_c)
        for w in range(cfg.nw):
            wm = wn_c == w
            rw = r_c[wm]
            cw = li_c[wm]
            vw = v_c[wm]
            deg = np.bincount(rw, minlength=cfg.npc)
            perm = np.argsort(-deg, kind="stable")
            slot_of = np.empty(cfg.npc, np.int64)
            slot_of[perm] = np.arange(cfg.npc)
            s_e = slot_of[rw]
            order = np.argsort(s_e, kind="stable")
            s_s = s_e[order]
            # j-rank within each slot group
            grp_start = np.searchsorted(s_s, np.arange(cfg.npc), side="left")
            jr = np.arange(s_s.size) - grp_start[s_s]
            bases = np.array(
                [stripe_base[(w, j)] for j in range(len(caps[w]))], np.int64
            )
            tok = bases[jr] + s_s
            g_stream[tok] = cw[order].astype(np.int16)
            v_stream[tok] = vw[order]
            sidx_all[w, :cfg.npc] = perm

        gw = np.zeros((128, total // 16), np.int16)
        vw_ = np.zeros((128, total // 128), np.float32)
        for (w, t0, ntok, segs) in lay["bins"]:
            gw[:, t0 // 16:(t0 + ntok) // 16] = _wrap16(g_stream[t0:t0 + ntok])
            vw_[:, t0 // 128:(t0 + ntok) // 128] = _wrap128(v_stream[t0:t0 + ntok])

        sidx = np.stack([_wrap16(sidx_all[w]) for w in range(cfg.nw)])

        xc = x[c * cfg.npc:(c + 1) * cfg.npc]
        if cfg.npc_pad > cfg.npc:
            xc = np.concatenate(
                [xc, np.zeros((cfg.npc_pad - cfg.npc, cfg.fin), np.float32)], axis=0
            )
        xT = np.ascontiguousarray(xc.T)

        in_maps.append({
            "xT": xT, "w1r": w1r, "b1r": b1r, "w2r": w2r, "b2r": b2r,
            "gidx": gw, "vals": vw_, "sidx": sidx,
        })
    return in_maps


_COMPILED2 = {}


def get_graph2(cfg: Cfg2, caps):
    key = (cfg, caps)
    if key not in _COMPILED2:
        nc = build_graph2(cfg, caps)
        nc.finalize()
        _COMPILED2[key] = nc
    return _COMPILED2[key]


def kernel(x, row, col, val, W1, b1, W2, b2, _cfg: Cfg2 = None, _want_results=False):
    if _cfg is None:
        _cfg = CFG2
    from concourse.bass_utils import run_bass_kernel_spmd

    cfg = _cfg
    row_a = np.asarray(row, np.int64)
    col_a = np.asarray(col, np.int64)
    defaults = DEFAULT_CAPS_SPLIT if cfg.split_ag else DEFAULT_CAPS
    caps = defaults if caps_fit(cfg, defaults, row_a, col_a) \
        else compute_caps(cfg, row_a, col_a)
    in_maps = preprocess2(cfg, caps, x, row, col, val, W1, b1, W2, b2)
    nc = get_graph2(cfg, caps)
    res = run_bass_kernel_spmd(nc, in_maps, core_ids=list(range(cfg.ncores)))
    outp = np.concatenate(
        [np.asarray(res.results[c]["out"]) for c in range(cfg.ncores)], axis=0
    ).astype(np.float32)
    if _want_results:
        return outp, res
    return outp


kernel2 = kernel



# revision 2
# speedup vs baseline: 7.3000x; 7.3000x over previous
"""v3 APPNP kernel: sharded striped-ELL spmm + XOR-butterfly allgather
via remote_dma_broadcast (replaces the ~34ms/iter collective_compute).

Per core c:
  - dense front on own 12.5K rows -> h slice (slot 0 of h_full).
  - h_full [8 slots x 12544 rows]: slot j holds slice c^j (XOR order, so
    SPMD addressing is static; gather indices compensate per core).
  - 10 iterations: gathers from h_full windows (window w = slots 2w,2w+1,
    int16 idx < 25088), striped-ELL accumulate on own dests, scatter-drain
    + alpha into nxt slot 0, then 3-step XOR butterfly exchange
    (partners c^1, c^2, c^4; physical tpb deltas 1, 2, 6) rebuilds all 8
    slots of nxt on every core.
  - out = slot 0 of the final h.

Exchange facts (HW-verified by probes):
  - remote_dma_broadcast with len-16 relative rdests: 1 DMA engine per
    slot, payload coherent. Cross-die logical partner c^4 needs physical
    Δtpb=6 (probe-verified bit map f(1)=1, f(2)=2, f(6)=4).
  - len-8 rdests split payloads across 2 engines with a swapped landing
    order on cross-die paths -> avoided.
  - tiny collectives are ~free; the bir_kernel_barrier prelude is cheap.
"""

from dataclasses import dataclass

import numpy as np


@dataclass(frozen=True)
class Cfg3:
    n: int = 100000
    fin: int = 512
    fhid: int = 256
    fout: int = 64
    alpha: float = 0.01
    k: int = 10
    ncores: int = 8
    nw: int = 4
    bin_tok: int = 12288
    xch: int = 4              # exchange chunks (cols of the 6272-col slice)

    @property
    def npc(self):
        return self.n // self.ncores          # 12500

    @property
    def npc_pad(self):
        return ((self.npc + 127) // 128) * 128  # 12544

    @property
    def win(self):
        return 2 * self.npc_pad               # 25088 rows per window

    @property
    def slot_cols(self):
        return self.npc_pad // 128            # 98

    @property
    def flat0(self):
        return self.npc_pad * self.fout // 128  # 6272 cols, slot-flat view


CFG3 = Cfg3()

# Shared static stripe caps (max over all (core, xor-pair-window) groups of
# the seed-0 instance); caps_fit3 verifies and compute_caps3 recomputes.
DEFAULT_CAPS3 = (12544, 12544, 12416, 12032, 11392, 10240, 8832, 7040,
                 5248, 3712, 2432, 1536, 896, 512, 384, 128, 128, 128,
                 128, 128, 128, 128, 128)

# physical tpb XOR-delta for logical partner c^(2^s) (probe-verified)
PHYS_DELTA = {1: 1, 2: 2, 4: 6}


def plan_layout3(cfg: Cfg3, caps):
    """Token-stream layout for one window (all 4 windows identical)."""
    stripes = []
    t = 0
    for j, cap in enumerate(caps):
        assert cap % 128 == 0
        stripes.append((j, t, cap))
        t += cap
    wlen = t
    bins = []
    pos = 0
    while pos < wlen:
        ntok = min(cfg.bin_tok, wlen - pos)
        segs = []
        for (j, base, scap) in stripes:
            lo = max(pos, base)
            hi = min(pos + ntok, base + scap)
            if lo < hi:
                segs.append((j, (lo - base) // 128, (hi - lo) // 128,
                             (lo - pos) // 128))
        bins.append((pos, ntok, segs))
        pos += ntok
    return {"wlen": wlen, "bins": bins}


def build_graph3(cfg: Cfg3, caps):
    import concourse.bacc as bacc
    import concourse.mybir as mybir
    import concourse.tile as tile

    f32 = mybir.dt.float32
    i16 = mybir.dt.int16
    KC = cfg.fin // 128
    MC = cfg.fhid // 128
    SC = cfg.slot_cols
    lay = plan_layout3(cfg, caps)
    wlen = lay["wlen"]
    total = cfg.nw * wlen
    F0 = cfg.flat0                       # 6272
    XC = F0 // cfg.xch                   # 1568 cols per exchange chunk
    assert F0 % cfg.xch == 0

    nc = bacc.Bacc("TRN2", num_devices=cfg.ncores)

    xT = nc.dram_tensor("xT", [cfg.fin, cfg.npc_pad], f32, kind="ExternalInput")
    w1r = nc.dram_tensor("w1r", [128, KC, MC, 128], f32, kind="ExternalInput")
    b1r = nc.dram_tensor("b1r", [128, MC], f32, kind="ExternalInput")
    w2r = nc.dram_tensor("w2r", [128, MC, cfg.fout], f32, kind="ExternalInput")
    b2r = nc.dram_tensor("b2r", [1, cfg.fout], f32, kind="ExternalInput")
    gidx = nc.dram_tensor("gidx", [128, total // 16], i16, kind="ExternalInput")
    vals = nc.dram_tensor("vals", [128, total // 128], f32, kind="ExternalInput")
    sidx = nc.dram_tensor(
        "sidx", [cfg.nw, 128, cfg.npc_pad // 16], i16, kind="ExternalInput"
    )
    out = nc.dram_tensor("out", [cfg.npc, cfg.fout], f32, kind="ExternalOutput")

    nrows = 8 * cfg.npc_pad
    h_a = nc.dram_tensor("h_a", [nrows, cfg.fout], f32, kind="Internal")
    h_b = nc.dram_tensor("h_b", [nrows, cfg.fout], f32, kind="Internal")

    # manual semaphores for the exchange protocol (python-int cumulative
    # targets; graph is fully unrolled so all targets are static)
    semA = nc.alloc_semaphore("xch_recv")    # remote data arrived
    semB = nc.alloc_semaphore("xch_sent")    # my broadcast data fully sent
    semP = nc.alloc_semaphore("xch_prep")    # descgen committed
    semS = nc.alloc_semaphore("xch_spill")   # spill dma done
    semL = nc.alloc_semaphore("xch_load")    # xbuf load done
    semR = nc.alloc_semaphore("xch_ready")   # partner drained its xbuf
    cnt = {"A": 0, "B": 0, "P": 0, "S": 0, "L": 0, "R": 0}

    def slot_flat(t, j):
        return t[j * cfg.npc_pad:(j + 1) * cfg.npc_pad, :] \
            .rearrange("a b -> (a b)").rearrange("(p q) -> p q", p=128)

    Pool = mybir.EngineType.Pool

    with tile.TileContext(nc) as tc:
        # ---------------- dense phase ----------------
        with tc.tile_pool(name="wpool", bufs=1) as wpool, \
             tc.tile_pool(name="xpool", bufs=2) as xpool, \
             tc.tile_pool(name="hpool", bufs=2) as hpool, \
             tc.tile_pool(name="opool", bufs=3) as opool, \
             tc.tile_pool(name="ppool", bufs=2, space="PSUM") as ppool, \
             tc.tile_pool(name="p2pool", bufs=2, space="PSUM") as p2pool:
            w1t = wpool.tile([128, KC, MC, 128], f32)
            nc.sync.dma_start(w1t[:], w1r[:])
            b1t = wpool.tile([128, MC], f32)
            nc.sync.dma_start(b1t[:], b1r[:])
            w2t = wpool.tile([128, MC, cfg.fout], f32)
            nc.sync.dma_start(w2t[:], w2r[:])
            b2t = wpool.tile([1, cfg.fout], f32)
            nc.sync.dma_start(b2t[:], b2r[:])
            onest = wpool.tile([1, 128], f32)
            nc.vector.memset(onest[:], 1.0)

            row0 = 0
            while row0 < cfg.npc:
                nsz = min(512, cfg.npc_pad - row0)
                xt = xpool.tile([128, KC, nsz], f32, tag="xt")
                nc.sync.dma_start(
                    xt[:],
                    xT[:, row0:row0 + nsz].rearrange("(kc p) n -> p kc n", p=128),
                )
                h1t = []
                for mc in range(MC):
                    ps = ppool.tile([128, nsz], f32, tag="ps")
                    for kc in range(KC):
                        nc.tensor.matmul(
                            ps[:], w1t[:, kc, mc, :], xt[:, kc, :],
                            start=(kc == 0), stop=(kc == KC - 1),
                        )
                    ht = hpool.tile([128, nsz], f32, tag=f"h1t{mc}")
                    nc.scalar.activation(
                        ht[:], ps[:], mybir.ActivationFunctionType.Relu,
                        bias=b1t[:, mc:mc + 1], scale=1.0,
                    )
                    h1t.append(ht)
                for ns in range(0, nsz, 128):
                    ssz = min(128, nsz - ns)
                    rows = min(ssz, cfg.npc - (row0 + ns))
                    if rows <= 0:
                        break
                    ps2 = p2pool.tile([128, cfg.fout], f32, tag="ps2")
                    for mc in range(MC):
                        nc.tensor.matmul(
                            ps2[:ssz, :], h1t[mc][:, ns:ns + ssz], w2t[:, mc, :],
                            start=(mc == 0), stop=False,
                        )
                    nc.tensor.matmul(
                        ps2[:ssz, :], onest[:, :ssz], b2t[:], start=False, stop=True
                    )
                    ho = opool.tile([128, cfg.fout], f32, tag="ho")
                    nc.scalar.activation(
                        ho[:ssz, :], ps2[:ssz, :], mybir.ActivationFunctionType.Copy
                    )
                    gr0 = row0 + ns
                    nc.sync.dma_start(h_a[gr0:gr0 + rows, :], ho[:rows, :])
                row0 += nsz

        # ---------------- propagation ----------------
        with tc.tile_pool(name="sxpool", bufs=1) as sxpool, \
             tc.tile_pool(name="apool", bufs=2) as apool, \
             tc.tile_pool(name="mpool", bufs=2) as mpool, \
             tc.tile_pool(name="ipool", bufs=6) as ipool, \
             tc.tile_pool(name="dqf", bufs=2) as dqf, \
             tc.tile_pool(name="xpool2", bufs=1) as xpool2:

            sidx_t = []
            for w in range(cfg.nw):
                st = sxpool.tile([128, cfg.npc_pad // 16], i16, tag=f"sidx{w}")
                nc.sync.dma_start(st[:], sidx[w])
                sidx_t.append(st)

            xbuf = xpool2.tile([128, 8, XC], f32)
            rbuf = xpool2.tile([128, 2], f32)
            rland = xpool2.tile([128, 2], f32)
            nc.vector.memset(rbuf[:], 1.0)

            def exchange(nxt, first):
                """Butterfly-allgather nxt slot0 -> slots 1..7 (all cores)."""
                xflat = xbuf[:].rearrange("p a b -> p (a b)")
                with tc.tile_critical(sync_engine=Pool):
                    if first:
                        nc.gpsimd.bir_kernel_barrier_wait(
                            [list(range(cfg.ncores))])
                    for ck in range(cfg.xch):
                        c0 = ck * XC
                        if not (first and ck == 0):
                            # partners drained their xbuf of the prev chunk
                            cnt["R"] += 3
                        nc.scalar.wait_ge(semR, cnt["R"])
                        nc.scalar.wait_ge(semB, cnt["B"])
                        nc.scalar.dma_start(
                            xbuf[:, 0, :], slot_flat(nxt, 0)[:, c0:c0 + XC]
                        ).then_inc(semL, 16)
                        cnt["L"] += 16
                        nc.gpsimd.wait_ge(semL, cnt["L"])
                        for s in range(3):
                            n = 1 << s
                            d = PHYS_DELTA[n]
                            half = n * XC // 2
                            for piece, slot in ((0, d), (1, d | 8)):
                                src = xflat[:, piece * half:(piece + 1) * half]
                                dst = xflat[:, n * XC + piece * half:
                                            n * XC + (piece + 1) * half]
                                rdests = [(0, d) if kk == slot else None
                                          for kk in range(16)]
                                nc.gpsimd.remote_dma_broadcast(
                                    dst, src, remote_sem=semA, local_sem=semB,
                                    rdests=rdests,
                                ).then_inc(semP, 1)
                                cnt["P"] += 1
                                cnt["B"] += 16
                            nc.gpsimd.wait_ge(semP, cnt["P"])
                            nc.gpsimd.trigger_dma(count=2)
                            cnt["A"] += 2
                            nc.gpsimd.wait_ge(semA, cnt["A"])
                        # spill received slots 1..7 to local HBM (ACT hwdge)
                        nc.scalar.wait_ge(semA, cnt["A"])
                        for j in range(1, 8):
                            nc.scalar.dma_start(
                                slot_flat(nxt, j)[:, c0:c0 + XC], xbuf[:, j, :]
                            ).then_inc(semS, 16)
                            cnt["S"] += 16
                        nc.gpsimd.wait_ge(semS, cnt["S"])
                        # tell my 3 partners my xbuf is reusable
                        rd = [None] * 16
                        for n in (1, 2, 4):
                            rd[PHYS_DELTA[n]] = (0, PHYS_DELTA[n])
                        nc.gpsimd.remote_dma_broadcast(
                            rland[:], rbuf[:], remote_sem=semR, local_sem=semB,
                            rdests=rd,
                        ).then_inc(semP, 1)
                        cnt["P"] += 1
                        cnt["B"] += 16
                        nc.gpsimd.wait_ge(semP, cnt["P"])
                        nc.gpsimd.trigger_dma(count=1)

            # h0 exchange (dense wrote h_a slot 0)
            exchange(h_a, first=True)

            for it in range(cfg.k):
                cur = h_a if it % 2 == 0 else h_b
                nxt = h_b if it % 2 == 0 else h_a

                # nxt slot0 = alpha * cur slot0
                for qc in range(2):
                    sl = slice(qc * (F0 // 2), (qc + 1) * (F0 // 2))
                    ht = dqf.tile([128, F0 // 2], f32, tag="alpha")
                    nc.sync.dma_start(ht[:], slot_flat(cur, 0)[:, sl])
                    nc.vector.tensor_scalar_mul(ht[:], ht[:], float(cfg.alpha))
                    nc.sync.dma_start(slot_flat(nxt, 0)[:, sl], ht[:])

                for w in range(cfg.nw):
                    acc = apool.tile([128, SC, cfg.fout], f32, tag="acc")
                    m0 = caps[0] // 128
                    if m0 < SC:
                        nc.vector.memset(acc[:, m0:, :], 0.0)
                    for (t0w, ntok, segs) in lay["bins"]:
                        t0 = w * wlen + t0w
                        nb = ntok // 128
                        gi = ipool.tile([128, cfg.bin_tok // 16], i16, tag="gi")
                        nc.sync.dma_start(
                            gi[:, :ntok // 16], gidx[:, t0 // 16:(t0 + ntok) // 16]
                        )
                        vt = ipool.tile([128, cfg.bin_tok // 128], f32, tag="vt")
                        nc.sync.dma_start(
                            vt[:, :nb], vals[:, t0 // 128:(t0 + ntok) // 128]
                        )
                        ms = mpool.tile(
                            [128, cfg.bin_tok // 128, cfg.fout], f32, tag="ms")
                        nc.gpsimd.dma_gather(
                            ms[:, :nb, :],
                            cur[w * cfg.win:(w + 1) * cfg.win, :],
                            gi[:, :ntok // 16],
                            ntok, ntok, cfg.fout,
                            single_packet=False,
                        )
                        nc.vector.tensor_mul(
                            ms[:, :nb, :], ms[:, :nb, :],
                            vt[:, :nb].unsqueeze(2).broadcast_to(
                                (128, nb, cfg.fout)),
                        )
                        for (j, a0, ncol, b0) in segs:
                            src = ms[:, b0:b0 + ncol, :]
                            dst = acc[:, a0:a0 + ncol, :]
                            if j == 0:
                                nc.vector.tensor_copy(dst, src)
                            else:
                                nc.vector.tensor_add(dst, dst, src)
                    # drain window w into nxt slot 0 (2 half-scatters)
                    half = cfg.npc_pad // 2
                    for hh in range(2):
                        lo = hh * half
                        nvalid = min(max(cfg.npc - lo, 0), half)
                        if nvalid == 0:
                            continue
                        nc.gpsimd.dma_scatter_add(
                            nxt[0:cfg.npc_pad, :],
                            acc[:, lo // 128:(lo + half) // 128, :],
                            sidx_t[w][:, lo // 16:(lo + half) // 16],
                            half, nvalid, cfg.fout,
                            single_packet=False,
                        )

                if it + 1 < cfg.k:
                    exchange(nxt, first=False)

            # final out = fin rows [0, 12500). Rows [0, 12416) move as an
            # aligned 128-partition flat block; the 84-row tail separately.
            fin = h_b if cfg.k % 2 == 1 else h_a
            r128 = (cfg.npc // 128) * 128                  # 12416
            fcols = r128 * cfg.fout // 128                 # 6208
            for qc in range(2):
                sl = slice(qc * (fcols // 2), (qc + 1) * (fcols // 2))
                ot = dqf.tile([128, fcols // 2], f32, tag="alpha")
                nc.sync.dma_start(
                    ot[:],
                    fin[0:r128, :].rearrange("a b -> (a b)")
                    .rearrange("(p q) -> p q", p=128)[:, sl])
                nc.sync.dma_start(
                    out[0:r128, :].rearrange("a b -> (a b)")
                    .rearrange("(p q) -> p q", p=128)[:, sl],
                    ot[:])
            tl = dqf.tile([128, cfg.fout], f32, tag="tail")
            nrem = cfg.npc - r128                          # 84
            nc.sync.dma_start(tl[:nrem, :], fin[r128:cfg.npc, :])
            nc.sync.dma_start(out[r128:cfg.npc, :], tl[:nrem, :])

    return nc


# ---------------------------------------------------------------------------
# Host preprocessing
# ---------------------------------------------------------------------------

def _wrap16(a):
    a = a.reshape(-1, 16).T
    return np.ascontiguousarray(np.tile(a, (8, 1)).astype(np.int16))


def _wrap128(a):
    return np.ascontiguousarray(a.reshape(-1, 128).T)


def compute_caps3(cfg: Cfg3, row, col):
    maxlen = 0
    tabs = []
    for c in range(cfg.ncores):
        rm = row // cfg.npc == c
        r_c = row[rm] - c * cfg.npc
        w_c = ((col[rm] // cfg.npc) ^ c) // 2
        for w in range(cfg.nw):
            deg = np.bincount(r_c[w_c == w], minlength=cfg.npc)
            mx = int(deg.max())
            tabs.append([int((deg > j).sum()) for j in range(mx)])
            maxlen = max(maxlen, mx)
    caps = []
    for j in range(maxlen):
        mm = max((t[j] if j < len(t) else 0) for t in tabs)
        caps.append(-(-mm // 128) * 128)
    return tuple(caps)


def caps_fit3(cfg: Cfg3, caps, row, col):
    try:
        if any(c > cfg.npc_pad or c <= 0 for c in caps):
            return False
        for c in range(cfg.ncores):
            rm = row // cfg.npc == c
            r_c = row[rm] - c * cfg.npc
            w_c = ((col[rm] // cfg.npc) ^ c) // 2
            for w in range(cfg.nw):
                deg = np.bincount(r_c[w_c == w], minlength=cfg.npc)
                mx = int(deg.max())
                if mx > len(caps):
                    return False
                nj = np.array([(deg > j).sum() for j in range(mx)])
                if (nj > np.array(caps[:mx])).any():
                    return False
        return True
    except Exception:
        return False


def preprocess3(cfg: Cfg3, caps, x, row, col, val, W1, b1, W2, b2):
    KC = cfg.fin // 128
    MC = cfg.fhid // 128
    lay = plan_layout3(cfg, caps)
    wlen = lay["wlen"]
    total = cfg.nw * wlen

    x = np.asarray(x, np.float32)
    row = np.asarray(row, np.int64)
    col = np.asarray(col, np.int64)
    val = np.asarray(val, np.float32)
    W1 = np.asarray(W1, np.float32)
    b1 = np.asarray(b1, np.float32)
    W2 = np.asarray(W2, np.float32)
    b2 = np.asarray(b2, np.float32)

    w1r = np.ascontiguousarray(W1.reshape(KC, 128, MC, 128).transpose(1, 0, 2, 3))
    b1r = np.ascontiguousarray(b1.reshape(MC, 128).T)
    w2r = np.ascontiguousarray(W2.reshape(MC, 128, cfg.fout).transpose(1, 0, 2))
    b2r = np.ascontiguousarray(b2.reshape(1, cfg.fout))

    stripe_base = {}
    t = 0
    for j, cap in enumerate(caps):
        stripe_base[j] = t
        t += cap
    assert t == wlen

    valp = val * np.float32(1.0 - cfg.alpha)
    in_maps = []
    for c in range(cfg.ncores):
        cm = row // cfg.npc == c
        r_c = row[cm] - c * cfg.npc
        col_c = col[cm]
        v_c = valp[cm]
        s_c = col_c // cfg.npc
        j_c = s_c ^ c
        w_all = j_c // 2
        li_all = (j_c % 2) * cfg.npc_pad + (col_c % cfg.npc)

        g_stream = np.zeros(total, np.int16)
        v_stream = np.zeros(total, np.float32)
        sidx_all = np.full((cfg.nw, cfg.npc_pad), -1, np.int64)

        for w in range(cfg.nw):
            wm = w_all == w
            rw = r_c[wm]
            cw = li_all[wm]
            vw = v_c[wm]
            deg = np.bincount(rw, minlength=cfg.npc)
            perm = np.argsort(-deg, kind="stable")
            slot_of = np.empty(cfg.npc, np.int64)
            slot_of[perm] = np.arange(cfg.npc)
            s_e = slot_of[rw]
            order = np.argsort(s_e, kind="stable")
            s_s = s_e[order]
            grp_start = np.searchsorted(s_s, np.arange(cfg.npc), side="left")
            jr = np.arange(s_s.size) - grp_start[s_s]
            bases = np.array([stripe_base[j] for j in range(len(caps))],
                             np.int64)
            tok = w * wlen + bases[jr] + s_s
            g_stream[tok] = cw[order].astype(np.int16)
            v_stream[tok] = vw[order]
            sidx_all[w, :cfg.npc] = perm

        gw = _wrap16(g_stream)
        vw_ = _wrap128(v_stream)
        sidx = np.stack([_wrap16(sidx_all[w]) for w in range(cfg.nw)])

        xc = x[c * cfg.npc:(c + 1) * cfg.npc]
        if cfg.npc_pad > cfg.npc:
            xc = np.concatenate(
                [xc, np.zeros((cfg.npc_pad - cfg.npc, cfg.fin), np.float32)],
                axis=0)
        xT = np.ascontiguousarray(xc.T)

        in_maps.append({
            "xT": xT, "w1r": w1r, "b1r": b1r, "w2r": w2r, "b2r": b2r,
            "gidx": gw, "vals": vw_, "sidx": sidx,
        })
    return in_maps


_COMPILED3 = {}


def get_graph3(cfg: Cfg3, caps):
    key = (cfg, caps)
    if key not in _COMPILED3:
        nc = build_graph3(cfg, caps)
        nc.finalize()
        _COMPILED3[key] = nc
    return _COMPILED3[key]


def kernel(x, row, col, val, W1, b1, W2, b2, _cfg: Cfg3 = None):
    if _cfg is None:
        _cfg = CFG3
    from concourse.bass_utils import run_bass_kernel_spmd

    cfg = _cfg
    row_a = np.asarray(row, np.int64)
    col_a = np.asarray(col, np.int64)
    caps = DEFAULT_CAPS3 if caps_fit3(cfg, DEFAULT_CAPS3, row_a, col_a) \
        else compute_caps3(cfg, row_a, col_a)
    in_maps = preprocess3(cfg, caps, x, row, col, val, W1, b1, W2, b2)
    nc = get_graph3(cfg, caps)
    res = run_bass_kernel_spmd(nc, in_maps, core_ids=list(range(cfg.ncores)))
    return np.concatenate(
        [np.asarray(res.results[c]["out"]) for c in range(cfg.ncores)], axis=0
    ).astype(np.float32)
